# revision 1
# baseline (speedup 1.0000x reference)
"""Fused transformer block (LN + fused QKV/FF proj + MQA attention + SwiGLU FF)
on 8 TRN2 NeuronCores — fp8-DoubleRow / fp16 version.

Sharding: DP2 x TP4 (core c -> batch c//4, shard c%4), feature-major layout.

Numerics:
  - proj runs at 16x scale in PSUM via fp8 DoubleRow 3-term scheme:
      16*xn@W ~= x16@W_hi (main, k-tile pairs)
               + [x_lo16@W_hi + x_hi@W_lo16] (cross, per k-tile)
               - 16*mu*colsum (rank-1 LN mean correction, K=2 DR matmul)
    where x16 = 16*x_hi exactly; the /16 and rstd fold into one fp16
    broadcast multiplier (rstd/16) applied at PSUM evacuation.
  - stats: sum from x16 (fp8-DR ones), sumsq from host-shipped fp8(x^2).
  - attention: q/k fp16 sims, es = fp8e4(exp(sim-4)) (shift cancels in
    softmax), AV in fp8 DoubleRow over kt pairs with fused denominator row,
    attn out + attn_out/ff_out projections in fp16.
"""

import numpy as np
import ml_dtypes

# ---- problem shapes (hardcoded) ----
B, N, D = 2, 2048, 1024
DH = 64
HEADS = 8
ATTN_INNER = HEADS * DH          # 512
FF_INNER = 4 * D                 # 4096
T = N
P = 128
TS = 512
NTS = T // TS                    # 4
NK = D // P                      # 8
NCORES = 8
TP = 4
LH = HEADS // TP                 # 2 local heads
FF_SH = FF_INNER // TP           # 1024
FSH = LH * DH + 2 * DH + 2 * FF_SH   # 2304 packed proj cols per core
NF = FSH // P                    # 18
NKT = T // P                     # 16 key-token tiles
NKP = NKT // 2                   # 8 key-tile pairs

_F8 = ml_dtypes.float8_e4m3
_F16 = np.float16

_STATE = {}


def _build_nc():
    from concourse import bacc
    import concourse.tile as tile
    from concourse.tile import add_dep_helper
    import concourse.mybir as mybir

    f8 = mybir.dt.float8e4
    f16 = mybir.dt.float16
    bf16 = mybir.dt.bfloat16
    f32 = mybir.dt.float32
    f32r = mybir.dt.float32r
    AF = mybir.ActivationFunctionType
    DR = mybir.MatmulPerfMode.DoubleRow

    nc = bacc.Bacc("TRN2", target_bir_lowering=False, debug=False)

    # partition-major layouts so grouped DMAs enumerate like SBUF tiles.
    # xt dim1 order: [x16[k] x8 | (x_lo16[k], x_hi[k]) x8 interleaved]
    xt_d = nc.dram_tensor("xt", [P, 24, T], f8, kind="ExternalInput")
    x2_d = nc.dram_tensor("x2", [P, NK, T], f8, kind="ExternalInput")
    # wt dim1 order: [W_hi[k] x8 | W_lo16[k] x8]
    wt_d = nc.dram_tensor("wt", [P, 16, FSH], f8, kind="ExternalInput")
    # mu-correction stationary rows: [partition(1), half(2), FSH]
    cmu_d = nc.dram_tensor("cmu", [1, 2, FSH], f8, kind="ExternalInput")
    wao_d = nc.dram_tensor("wao", [P, D], f16, kind="ExternalInput")
    wfo_d = nc.dram_tensor("wfo", [NK, P, D], f16, kind="ExternalInput")
    yT_d = nc.dram_tensor("yT", [D, T], f16, kind="ExternalOutput")

    with tile.TileContext(nc) as tc:
        with (
            tc.tile_pool(name="cp", bufs=1) as cp,
            tc.tile_pool(name="wp", bufs=1) as wp,
            tc.tile_pool(name="acts", bufs=1) as acts,
            tc.tile_pool(name="rows", bufs=2) as rows,
            tc.tile_pool(name="tmp", bufs=3) as tmp,
            tc.tile_pool(name="ps", bufs=1, space="PSUM") as ps,
        ):
            # ---- constants ----
            ones2 = cp.tile([P, 2, 64], f8)      # DR stats lhsT (all ones;
                                                 # M=64 is the min legal DR
                                                 # stationary tile, row 0 read)
            nc.vector.memset(ones2, 1.0)
            ones_row_r = cp.tile([1, P], f32r)
            nc.vector.memset(ones_row_r.bitcast(f32), 1.0)
            ones_hi = cp.tile([P, 64], f32r)
            nc.vector.memset(ones_hi.bitcast(f32), 1.0)
            eps_t = cp.tile([1, 1], f32)
            nc.vector.memset(eps_t, 1e-5)
            zero_col = cp.tile([P, 1], f32)
            nc.vector.memset(zero_col, 0.0)
            nbias4 = cp.tile([P, 1], f32)
            nc.vector.memset(nbias4, -4.0)
            # warm ACT tables before bulk DMAs queue up
            warm_o = cp.tile([1, 4], f32)
            for wi, fn in enumerate((AF.Silu, AF.Exp, AF.Sqrt)):
                nc.scalar.activation(warm_o[0:1, wi:wi + 1],
                                     eps_t, fn, bias=zero_col[0:1, :])

            # ---- persistent activations ----
            q8_sb = acts.tile([P, 2, T], f8)   # q both heads, duplicated
            k8_sb = acts.tile([64, 2, T], f8)  # k (hi, lo) head-0 rows
            k28_sb = acts.tile([P, 2, T], f8)  # k replica at partitions 64+
            v_sb = acts.tile([P, T], f16)      # v at partitions 64-127
            h_sb = [acts.tile([P, T], f16, name=f"h{j}") for j in range(NK)]
            out_sb = acts.tile([P, T], f16)    # attention out, both heads
            rstd_b = acts.tile([P, T], f16)    # rstd/16 broadcast
            v_aug = [acts.tile([P, 2, P], f8, name=f"va{kp}")
                     for kp in range(NKP)]
            for kp in range(NKP):
                nc.vector.memset(v_aug[kp], 0.0)
                nc.vector.memset(v_aug[kp][:, 0, 64:65], 1.0)
                nc.vector.memset(v_aug[kp][:, 1, 64:65], 1.0)
            cmu_t = wp.tile([1, 2, FSH], f8)

            with (
                tc.tile_pool(name="xp", bufs=1) as xp,
                tc.tile_pool(name="x2p", bufs=2) as x2p,
            ):
                xt = xp.tile([P, 24, T], f8)
                wt = xp.tile([P, 16, FSH], f8)

                # ---- grouped loads (few HWDGE triggers; each trigger
                # costs ~625ns of serialized queue overhead) ----
                xh1 = slice(0, T // 2)
                xh2 = slice(T // 2, T)
                nc.sync.dma_start(out=xt[:, 0:2, xh1], in_=xt_d[:, 0:2, xh1])
                nc.sync.dma_start(out=xt[:, 2:8, xh1], in_=xt_d[:, 2:8, xh1])
                x2s = {}
                def load_x2(ts):
                    x2t = x2p.tile([P, NK, TS], f8, tag="x2",
                                   name=f"x2_{ts}")
                    col = slice(ts * TS, (ts + 1) * TS)
                    nc.sync.dma_start(out=x2t[:, :, :], in_=x2_d[:, :, col])
                    x2s[ts] = x2t
                load_x2(0)
                nc.sync.dma_start(out=cmu_t[:, :, :], in_=cmu_d[:, :, :])
                load_x2(1)
                nc.sync.dma_start(out=xt[:, 8:16, xh1], in_=xt_d[:, 8:16, xh1])
                nc.sync.dma_start(out=xt[:, 16:24, xh1],
                                  in_=xt_d[:, 16:24, xh1])
                WQ = FSH // 4
                nc.sync.dma_start(out=wt[:, :, 0:WQ], in_=wt_d[:, :, 0:WQ])
                for qw in range(1, 4):
                    qc = slice(qw * WQ, (qw + 1) * WQ)
                    nc.sync.dma_start(out=wt[:, :, qc], in_=wt_d[:, :, qc])
                for r0 in range(0, 8, 4):
                    nc.sync.dma_start(out=xt[:, r0:r0 + 4, xh2],
                                      in_=xt_d[:, r0:r0 + 4, xh2])
                for r0 in range(8, 24, 4):
                    nc.sync.dma_start(out=xt[:, r0:r0 + 4, xh2],
                                      in_=xt_d[:, r0:r0 + 4, xh2])

                # ---- LayerNorm statistics per token-slice ----
                stat_ps = {}

                def emit_stats(ts):
                    col = slice(ts * TS, (ts + 1) * TS)
                    ps_s = ps.tile([P, TS], f32, tag="pav", bufs=2,
                                   name=f"ps_s{ts}")
                    for j in range(4):
                        nc.tensor.matmul(ps_s[0:64, :], lhsT=ones2,
                                         rhs=xt[:, 2 * j:2 * j + 2, col],
                                         start=(j == 0), stop=(j == 3),
                                         perf_mode=DR)
                    ps_s2 = ps.tile([P, TS], f32, tag="pav", bufs=2,
                                     name=f"ps_s2{ts}")
                    x2t = x2s.pop(ts)
                    for j in range(4):
                        nc.tensor.matmul(ps_s2[0:64, :], lhsT=ones2,
                                         rhs=x2t[:, 2 * j:2 * j + 2, :],
                                         start=(j == 0), stop=(j == 3),
                                         perf_mode=DR)
                    stat_ps[ts] = (ps_s, ps_s2)

                murows = {}

                def emit_rows(ts):
                    # ps_s = 16*sum(x); ps_s2 = sum(x^2)
                    col = slice(ts * TS, (ts + 1) * TS)
                    ps_s, ps_s2 = stat_ps.pop(ts)
                    # all PSUM readers on ACT (Square/Copy live in every
                    # table set): stats psums release without touching the
                    # backlogged DVE queue
                    sqsb = rows.tile([1, TS], f16, tag="sqsb")
                    nc.scalar.activation(sqsb, ps_s[0:1, :], AF.Square,
                                         scale=1.0 / (16.0 * D ** 0.5))
                    s2sb = rows.tile([1, TS], f16, tag="s2sb")
                    nc.scalar.activation(s2sb, ps_s2[0:1, :], AF.Copy)
                    mt = rows.tile([1, 2, TS], f8, tag="mt", name=f"mt{ts}")
                    nc.gpsimd.memset(mt[0:1, 1, :], 0.0)
                    nc.scalar.activation(mt[0:1, 0, :], ps_s[0:1, :],
                                         AF.Copy, scale=1.0 / D)
                    murows[ts] = mt
                    # negvarD = D*mu^2 - sum(x^2) = -D*var
                    negvar = rows.tile([1, TS], f32, tag="negvar")
                    nc.vector.scalar_tensor_tensor(
                        negvar, sqsb, 1.0, s2sb,
                        op0=mybir.AluOpType.mult,
                        op1=mybir.AluOpType.subtract)
                    std = rows.tile([1, TS], f32, tag="std")
                    nc.scalar.activation(std, negvar, AF.Sqrt, bias=eps_t,
                                         scale=-1.0 / D)
                    rr = rows.tile([1, TS], f32r, tag="rr")
                    with nc.allow_low_precision(reason="rstd broadcast"):
                        nc.vector.reciprocal(rr, std)
                    # PE broadcast (DMA-free): tiny DMAs would queue behind
                    # bulk loads on the serial DMA device and stall the slice
                    prs = ps.tile([P, TS], f32, tag="pp", bufs=4,
                                  name=f"prs{ts}")
                    nc.tensor.matmul(prs, lhsT=ones_row_r[0:1, :],
                                     rhs=rr, start=True, stop=True)
                    nc.vector.tensor_scalar_mul(rstd_b[:, col], prs,
                                                1.0 / 16.0)


                gate = {}
                _STATE_VA = {}

                # ---- fused projection ----
                # packed col order: [q(128) | kv(128) | (gate_j, ffx_j) x 8]
                def emit_proj(ts):
                    col = slice(ts * TS, (ts + 1) * TS)
                    mt = murows[ts]
                    cur_sl = None
                    for fi in range(NF):
                        fc = slice(fi * P, (fi + 1) * P)
                        pp = ps.tile([P, TS], f32, tag="pp", bufs=4,
                                     name=f"pp{ts}_{fi}")
                        for j in range(4):
                            nc.tensor.matmul(
                                pp, lhsT=wt[:, 2 * j:2 * j + 2, fc],
                                rhs=xt[:, 2 * j:2 * j + 2, col],
                                start=(j == 0), stop=False, perf_mode=DR)
                        for k in range(NK):
                            nc.tensor.matmul(
                                pp, lhsT=wt[:, k:k + 9:8, fc],
                                rhs=xt[:, 8 + 2 * k:10 + 2 * k, col],
                                start=False, stop=False, perf_mode=DR)
                        nc.tensor.matmul(
                            pp, lhsT=cmu_t[:, :, fc], rhs=mt[:, :, :],
                            start=False, stop=True, perf_mode=DR)
                        if fi == 0:
                            nc.vector.tensor_mul(q8_sb[:, 0, col], pp,
                                                 rstd_b[:, col])
                            nc.vector.tensor_copy(q8_sb[:, 1, col],
                                                  q8_sb[:, 0, col])
                        elif fi == 1:
                            ktmp = tmp.tile([64, TS], bf16, tag="ktmp")
                            nc.vector.tensor_mul(ktmp, pp[0:64, :],
                                                 rstd_b[0:64, col])
                            nc.vector.tensor_copy(k8_sb[:, 0, col], ktmp)
                            nc.vector.scalar_tensor_tensor(
                                k8_sb[:, 1, col], k8_sb[:, 0, col], -1.0,
                                ktmp, op0=mybir.AluOpType.mult,
                                op1=mybir.AluOpType.add)
                            nc.vector.tensor_mul(v_sb[64:128, col],
                                                 pp[64:128, :],
                                                 rstd_b[64:128, col])
                            nc.sync.dma_start(out=k28_sb[64:128, :, col],
                                              in_=k8_sb[:, :, col])
                        elif fi % 2 == 0:  # gate_j
                            gg = tmp.tile([P, TS], bf16, tag="gg")
                            nc.vector.tensor_mul(gg, pp, rstd_b[:, col])
                            sl = tmp.tile([P, TS], f16, tag="sl")
                            nc.scalar.activation(sl, gg, AF.Silu,
                                                 bias=zero_col)
                            cur_sl = sl
                        else:  # ffx_j
                            j = (fi - 3) // 2
                            u2 = tmp.tile([P, TS], f16, tag="u2")
                            nc.vector.tensor_mul(u2, cur_sl, rstd_b[:, col])
                            hmul = nc.vector.tensor_mul(h_sb[j][:, col],
                                                        u2, pp)
                            if ts == 0:
                                gate[j] = hmul
                    # v -> token-major via fp16 transpose DMA, then fp8 cast
                    for kt in range(ts * (TS // P), (ts + 1) * (TS // P)):
                        kp, half = kt // 2, kt % 2
                        if half == 0:
                            va16 = tmp.tile([P, 2, 64], f16, tag="va16",
                                            name=f"va16_{kp}")
                            _STATE_VA[kp] = va16
                        nc.sync.dma_start(
                            out=_STATE_VA[kp][:, half, :],
                            in_=v_sb[64:128, kt * P:(kt + 1) * P],
                            transpose=True)
                        if half == 1:
                            nc.vector.tensor_copy(v_aug[kp][:, :, 0:64],
                                                  _STATE_VA.pop(kp))

                emit_stats(0)
                emit_rows(0)
                emit_stats(1)
                emit_rows(1)
                emit_proj(0)
                load_x2(2)
                emit_stats(2)
                emit_rows(2)
                emit_proj(1)
                load_x2(3)
                emit_stats(3)
                emit_rows(3)
                emit_proj(2)
                emit_proj(3)
                # output-side weights deferred behind proj slice 1
                wao_sb = wp.tile([P, D], f16)
                w_in = nc.gpsimd.dma_start(out=wao_sb, in_=wao_d[:, :])
                add_dep_helper(w_in.ins, gate[7].ins,
                               reason="defer wao load")
                wfo_sb = []
                for k in range(NK):
                    t_ = wp.tile([P, D], f16, name=f"wfo{k}")
                    w_in = nc.gpsimd.dma_start(out=t_, in_=wfo_d[k][:, :])
                    add_dep_helper(w_in.ins, gate[k].ins,
                                   reason="defer wfo load")
                    wfo_sb.append(t_)

            # xp closed: reuse SBUF for attention pipeline.
            with (
                tc.tile_pool(name="esp", bufs=14) as esp,
                tc.tile_pool(name="atmp", bufs=3) as atmp,
                tc.tile_pool(name="yp", bufs=4) as yp,
                tc.tile_pool(name="yffp", bufs=1) as yffp,
            ):
                es_store = {}
                pavs = {}
                y_chains = []

                yff_sb = [yffp.tile([P, TS], f16, name=f"yff{d}")
                          for d in range(NK)]

                def y_ff_chain_gen(tsq, d):
                    qcol = slice(tsq * TS, (tsq + 1) * TS)
                    py = ps.tile([P, TS], f32, tag="pp", bufs=4,
                                 name=f"pyf{tsq}_{d}")
                    for k in range(NK):
                        nc.tensor.matmul(
                            py, lhsT=wfo_sb[k][:, d * P:(d + 1) * P],
                            rhs=h_sb[k][:, qcol],
                            start=(k == 0), stop=(k == NK - 1))
                        yield
                    nc.vector.tensor_copy(yff_sb[d], py)

                def y_attn_chain_gen(tsq, d):
                    qcol = slice(tsq * TS, (tsq + 1) * TS)
                    pa = ps.tile([P, TS], f32, tag="pp", bufs=4,
                                 name=f"pya{tsq}_{d}")
                    nc.tensor.matmul(pa, lhsT=wao_sb[:, d * P:(d + 1) * P],
                                     rhs=out_sb[:, qcol],
                                     start=True, stop=True)
                    yield
                    y_sb = yp.tile([P, TS], f16, tag="ysb",
                                   name=f"ysba{tsq}_{d}")
                    nc.vector.scalar_tensor_tensor(
                        y_sb, pa, 1.0, yff_sb[d],
                        op0=mybir.AluOpType.mult,
                        op1=mybir.AluOpType.add)
                    nc.sync.dma_start(out=yT_d[d * P:(d + 1) * P, qcol],
                                      in_=y_sb)

                def y_chain_gen(tsq, d):
                    qcol = slice(tsq * TS, (tsq + 1) * TS)
                    py = ps.tile([P, TS], f32, tag="pp", bufs=4,
                                 name=f"py{tsq}_{d}")
                    for k in range(NK):
                        nc.tensor.matmul(
                            py, lhsT=wfo_sb[k][:, d * P:(d + 1) * P],
                            rhs=h_sb[k][:, qcol],
                            start=(k == 0), stop=False)
                        yield
                    nc.tensor.matmul(
                        py, lhsT=wao_sb[:, d * P:(d + 1) * P],
                        rhs=out_sb[:, qcol], start=False, stop=True)
                    y_sb = yp.tile([P, TS], f16, tag="ysb",
                                   name=f"ysb{tsq}_{d}")
                    nc.vector.tensor_copy(y_sb, py)
                    nc.sync.dma_start(out=yT_d[d * P:(d + 1) * P, qcol],
                                      in_=y_sb)

                def y_step(n):
                    done = 0
                    while done < n and y_chains:
                        try:
                            next(y_chains[0])
                        except StopIteration:
                            y_chains.pop(0)
                        done += 1

                def emit_sims(tsq, kt):
                    qcol = slice(tsq * TS, (tsq + 1) * TS)
                    kcols = slice(kt * P, (kt + 1) * P)
                    kp, half = kt // 2, kt % 2
                    for h in range(LH):
                        psim = ps.tile([P, TS], f32, tag="psim", bufs=2,
                                       name=f"psim{tsq}_{h}_{kt}")
                        if h == 0:
                            nc.tensor.matmul(psim,
                                             lhsT=k8_sb[:, :, kcols],
                                             rhs=q8_sb[0:64, :, qcol],
                                             start=True, stop=True,
                                             perf_mode=DR)
                        else:
                            nc.tensor.matmul(psim,
                                             lhsT=k28_sb[64:128, :, kcols],
                                             rhs=q8_sb[64:128, :, qcol],
                                             start=True, stop=True,
                                             perf_mode=DR)
                        if half == 0:
                            es_store[(tsq, h, kp)] = esp.tile(
                                [P, 2, TS], f8, tag="es",
                                name=f"es{tsq}_{h}_{kp}")
                        nc.scalar.activation(es_store[(tsq, h, kp)][:, half, :],
                                             psim, AF.Exp, bias=nbias4)

                def av_mm(tsq, h, kp):
                    if kp == 0:
                        pavs[(tsq, h)] = ps.tile([P, TS], f32, tag="pav",
                                                 bufs=2, name=f"pav{tsq}_{h}")
                    nc.tensor.matmul(
                        pavs[(tsq, h)], lhsT=v_aug[kp][:, :, :],
                        rhs=es_store.pop((tsq, h, kp)),
                        start=(kp == 0), stop=(kp == NKP - 1),
                        perf_mode=DR)

                def emit_av_epilogue(tsq, h):
                    b = tsq * LH + h
                    qcol = slice(tsq * TS, (tsq + 1) * TS)
                    pav = pavs.pop((tsq, h))
                    rec64 = atmp.tile([P, TS], mybir.dt.float32r,
                                      tag="rec64")
                    with nc.allow_low_precision(reason="1/denom broadcast"):
                        nc.vector.reciprocal(rec64[64:65, :],
                                             pav[64:65, :])
                    pB = ps.tile([64, TS], f32, tag="pp", bufs=4,
                                 name=f"pB{b}")
                    nc.tensor.matmul(pB, lhsT=ones_hi[64:65, :],
                                     rhs=rec64[64:65, :],
                                     start=True, stop=True)
                    rb = atmp.tile([64, TS], f32, tag="rb")
                    nc.vector.tensor_copy(rb, pB)
                    if h == 0:
                        nc.vector.tensor_mul(out_sb[0:64, qcol],
                                             pav[0:64, :], rb)
                    else:
                        oh1 = atmp.tile([64, TS], f16, tag="oh1")
                        nc.vector.tensor_mul(oh1, pav[0:64, :], rb)
                        nc.sync.dma_start(out=out_sb[64:128, qcol], in_=oh1)
                        if tsq == 0:
                            y_chains.extend(y_attn_chain_gen(0, d)
                                            for d in range(NK))
                            y_chains.extend(y_ff_chain_gen(1, d)
                                            for d in range(NK))
                        elif tsq == 1:
                            y_chains.extend(y_attn_chain_gen(1, d)
                                            for d in range(NK))
                        else:
                            y_chains.extend(y_chain_gen(tsq, d)
                                            for d in range(NK))

                # slot 0: sims, with slice-0 ff chains as PE filler
                y_chains.extend(y_ff_chain_gen(0, d) for d in range(NK))
                for kt in range(NKT):
                    emit_sims(0, kt)
                    y_step(2)
                for bslot in range(1, NTS):
                    for kt in range(NKT):
                        emit_sims(bslot, kt)
                        if kt % 2 == 1:
                            kp = kt // 2
                            av_mm(bslot - 1, 0, kp)
                            if kp == NKP - 1:
                                emit_av_epilogue(bslot - 1, 0)
                            av_mm(bslot - 1, 1, kp)
                        y_step(3)
                    emit_av_epilogue(bslot - 1, 1)
                for kp in range(NKP):
                    av_mm(NTS - 1, 0, kp)
                    av_mm(NTS - 1, 1, kp)
                    y_step(3)
                emit_av_epilogue(NTS - 1, 0)
                emit_av_epilogue(NTS - 1, 1)
                y_step(1 << 30)

    nc.compile()
    return nc


def _get_nc():
    if "nc" not in _STATE:
        _STATE["nc"] = _build_nc()
    return _STATE["nc"]


def _f8(a):
    return np.asarray(a, dtype=_F8)


def _prep_inputs(x, gamma, w_fused, w_attn_out, w_ff_out):
    """Host-side shard packing + fp8/fp16 quantization."""
    x = np.asarray(x, dtype=np.float32)
    gamma = np.asarray(gamma, dtype=np.float32)
    w_fused = np.asarray(w_fused, dtype=np.float32)
    w_attn_out = np.asarray(w_attn_out, dtype=np.float32)
    w_ff_out = np.asarray(w_ff_out, dtype=np.float32)

    wf = w_fused * gamma[:, None]
    wf = wf.copy()
    wf[:, :ATTN_INNER] *= DH ** -0.5

    q_blk = wf[:, :ATTN_INNER]
    k_blk = wf[:, ATTN_INNER:ATTN_INNER + DH]
    v_blk = wf[:, ATTN_INNER + DH:ATTN_INNER + 2 * DH]
    ffx_blk = wf[:, ATTN_INNER + 2 * DH:ATTN_INNER + 2 * DH + FF_INNER]
    gate_blk = wf[:, ATTN_INNER + 2 * DH + FF_INNER:]

    # x tensors per batch
    xts = []
    x2s = []
    for b in range(B):
        xT = np.ascontiguousarray(x[b].T)                      # [D, T]
        x_hi8 = _f8(xT)
        x_hi = x_hi8.astype(np.float32)
        x16 = _f8(16.0 * x_hi)                                 # exact shift
        x_lo16 = _f8(16.0 * (xT - x_hi))
        xt = np.empty((24, P, T), dtype=_F8)
        for k in range(NK):
            rs = slice(k * P, (k + 1) * P)
            xt[k] = x16[rs]
            xt[8 + 2 * k] = x_lo16[rs]
            xt[9 + 2 * k] = x_hi8[rs]
        xts.append(np.ascontiguousarray(xt.transpose(1, 0, 2)))
        x2 = _f8(xT * xT).reshape(NK, P, T)
        x2s.append(np.ascontiguousarray(x2.transpose(1, 0, 2)))

    in_maps = []
    for c in range(NCORES):
        b, s = divmod(c, TP)
        cols = [q_blk[:, P * s:P * s + P], k_blk, v_blk]
        for j in range(NK):
            cols.append(gate_blk[:, FF_SH * s + j * P: FF_SH * s + (j + 1) * P])
            cols.append(ffx_blk[:, FF_SH * s + j * P: FF_SH * s + (j + 1) * P])
        wf_c = np.concatenate(cols, axis=1)                    # [D, FSH] f32
        w_hi8 = _f8(wf_c)
        w_hi = w_hi8.astype(np.float32)
        w_lo16 = _f8(16.0 * (wf_c - w_hi))
        wt = np.empty((16, P, FSH), dtype=_F8)
        for k in range(NK):
            rs = slice(k * P, (k + 1) * P)
            wt[k] = w_hi8[rs]
            wt[8 + k] = w_lo16[rs]
        wt = np.ascontiguousarray(wt.transpose(1, 0, 2))
        cs = wf_c.sum(0)                                       # [FSH] f32
        cs_hi = _f8(cs).astype(np.float32)
        cmu = np.zeros((1, 2, FSH), dtype=_F8)
        cmu[0, 0] = _f8(-cs_hi)
        cmu[0, 1] = _f8(-cs_hi / 16.0)
        wao_c = np.ascontiguousarray(
            w_attn_out[P * s:P * s + P, :]).astype(_F16)
        wfo_c = np.ascontiguousarray(
            w_ff_out[FF_SH * s:FF_SH * (s + 1), :]).reshape(
                NK, P, D).astype(_F16)
        in_maps.append({"xt": xts[b], "x2": x2s[b], "wt": wt, "cmu": cmu,
                        "wao": wao_c, "wfo": wfo_c})
    return in_maps


def kernel(x, gamma, w_fused, w_attn_out, w_ff_out):
    import time
    from concourse.bass_utils import run_bass_kernel_spmd

    nc = _get_nc()
    in_maps = _prep_inputs(x, gamma, w_fused, w_attn_out, w_ff_out)

    t0 = time.perf_counter()
    res = run_bass_kernel_spmd(nc, in_maps, core_ids=list(range(NCORES)))
    t1 = time.perf_counter()
    _STATE["last_wall_ns"] = (t1 - t0) * 1e9

    y = np.empty((B, N, D), dtype=np.float32)
    for b in range(B):
        acc = res.results[b * TP]["yT"].astype(np.float32)
        for s in range(1, TP):
            acc = acc + res.results[b * TP + s]["yT"].astype(np.float32)
        y[b] = acc.T
    return y

